# revision 37
# baseline (speedup 1.0000x reference)
"""Trainium2 Bass kernel for a dense transformer block (pre-LN attention + MLP).

Reference computation (B=4, N=2048, C=1024, H=4096, 16 heads, fp32):
    q = LN(x) @ wq + bq ; k/v = LN(x+pos) @ w{k,v} + b{k,v}
    attn = softmax(q k^T / sqrt(hd)) @ v ; h = x + attn @ wp + bp
    out = h + leaky_relu(LN(h) @ w1 + b1, 0.1) @ w2 + b2

Sharding: 8 cores; core c handles batch c//2, query-token half c%2. K/V
for the full 2048-token sequence are recomputed per core pair, so no
collectives are needed. All matmuls run in fp32r (full PE rate at free
dim 512, ~1e-4 rms error). LayerNorm runs in token-major layout
(per-partition stats on DVE/ACT), activations are PE-transposed into
C-major layout for the projections, LN gamma is folded into the weights
host-side and LN beta + biases fold into per-output constant rows.
K^T, Vtilde (V plus a ones-column that accumulates the softmax
denominator during the P@V matmul) and attn^T stage through DRAM.
"""

import numpy as np
from contextlib import ExitStack

import concourse.bass as bass
import concourse.bacc as bacc
import concourse.tile as tile
from concourse import mybir
from concourse.bass_utils import run_bass_kernel_spmd
from concourse.masks import make_identity

F32 = mybir.dt.float32
F32R = mybir.dt.float32r
AF = mybir.ActivationFunctionType

B, N, C, H, HEADS = 4, 2048, 1024, 4096, 16
HD = C // HEADS            # 64
TQ = N // 2                # query tokens per core = 1024
EPS = 1e-5
SCALE = float(HD) ** -0.5  # 1/8
P = 128
NCORES = 8

NT_KV = N // P             # 16 token tiles (kv side)
NT_Q = TQ // P             # 8 token tiles (q side)
NC_C = C // P              # 8 channel tiles
NO_C = C // 512            # 2 output col tiles of 512
NJ_H = H // P              # 32


def _ln_stats(nc, pool, x_tile, eps_tile):
    """Return (r, negmr) = (rsqrt(var+eps), -mean*r) as [P,1] fp32 tiles."""
    stats = pool.tile([P, 2, 6], F32, tag="bn_stats", name="bn_stats")
    for sg in range(2):
        nc.vector.bn_stats(stats[:, sg, :], x_tile[:, sg * 512:(sg + 1) * 512])
    mv = pool.tile([P, 2], F32, tag="bn_mv", name="bn_mv")
    nc.vector.bn_aggr(mv, stats)
    r = pool.tile([P, 1], F32, tag="ln_r", name="ln_r")
    negmr = pool.tile([P, 1], F32, tag="ln_negmr", name="ln_negmr")
    nc.scalar.activation(r, mv[:, 1:2], AF.Sqrt, bias=eps_tile)
    nc.vector.reciprocal(r, r)
    nc.vector.tensor_mul(negmr, mv[:, 0:1], r)
    nc.vector.tensor_scalar_mul(negmr, negmr, -1.0)
    return r, negmr


def build_program():
    nc = bacc.Bacc("TRN2", target_bir_lowering=False, debug=False)

    xb = nc.dram_tensor("xb", [N, C], F32, kind="ExternalInput")
    posd = nc.dram_tensor("pos", [N, C], F32, kind="ExternalInput")
    xq = nc.dram_tensor("xq", [TQ, C], F32, kind="ExternalInput")
    wq = nc.dram_tensor("wq", [C, C], F32, kind="ExternalInput")
    wk = nc.dram_tensor("wk", [C, C], F32, kind="ExternalInput")
    wv = nc.dram_tensor("wv", [C, C], F32, kind="ExternalInput")
    wp = nc.dram_tensor("wp", [C, C], F32, kind="ExternalInput")
    w1 = nc.dram_tensor("w1", [C, H], F32, kind="ExternalInput")
    w2 = nc.dram_tensor("w2", [H, C], F32, kind="ExternalInput")
    cq = nc.dram_tensor("cq", [C], F32, kind="ExternalInput")
    ck = nc.dram_tensor("ck", [C], F32, kind="ExternalInput")
    cv = nc.dram_tensor("cv", [C], F32, kind="ExternalInput")
    cp = nc.dram_tensor("cp", [C], F32, kind="ExternalInput")
    c1 = nc.dram_tensor("c1", [H], F32, kind="ExternalInput")
    c2 = nc.dram_tensor("c2", [C], F32, kind="ExternalInput")
    out = nc.dram_tensor("out", [TQ, C], F32, kind="ExternalOutput")
    # DRAM staging (fp32r bits; re-rounded by a copy after reload)
    vsp = nc.dram_tensor("vspill", [N, HEADS * 65], F32R, kind="Internal")
    ksp = nc.dram_tensor("kspill", [C, N], F32R, kind="Internal")
    qsp = nc.dram_tensor("qspill", [C, TQ], F32R, kind="Internal")

    xb_t = xb.ap().rearrange("(t p) c -> t p c", p=P)
    pos_t = posd.ap().rearrange("(t p) c -> t p c", p=P)
    xq_t = xq.ap().rearrange("(t p) c -> t p c", p=P)
    vsp_t = vsp.ap().rearrange("(t p) c -> t p c", p=P)
    ksp_t = ksp.ap().rearrange("(ct p) t -> ct p t", p=P)
    qsp_t = qsp.ap().rearrange("(ct p) t -> ct p t", p=P)

    with tile.TileContext(nc) as tc, ExitStack() as ctx:
        const = ctx.enter_context(tc.tile_pool(name="const", bufs=1))
        stat = ctx.enter_context(tc.tile_pool(name="stat", bufs=4))
        ld = ctx.enter_context(tc.tile_pool(name="ld", bufs=2))
        evict = ctx.enter_context(tc.tile_pool(name="evict", bufs=2))

        ident_f32 = const.tile([P, P], F32)
        make_identity(nc, ident_f32)
        ident = const.tile([P, P], F32R)
        nc.vector.tensor_copy(ident, ident_f32)
        eps_tile = const.tile([P, 1], F32)
        nc.vector.memset(eps_tile, EPS)
        ones16 = const.tile([P, 16], F32)
        nc.vector.memset(ones16, 1.0)

        # per-partition bias columns
        def col_const(src, n_tiles, name):
            t = const.tile([P, n_tiles], F32, tag=name, name=name)
            nc.sync.dma_start(t, bass.AP(tensor=src, offset=0, ap=[[1, P], [P, n_tiles]]))
            return t

        cq_sb = col_const(cq, NC_C, "cq_sb")
        ck_sb = col_const(ck, NC_C, "ck_sb")
        c1_sb = col_const(c1, NJ_H, "c1_sb")
        c1a_sb = const.tile([P, NJ_H], F32, tag="c1a_sb", name="c1a_sb")
        nc.vector.tensor_scalar_mul(c1a_sb, c1_sb, 0.45)
        c1b_sb = const.tile([P, NJ_H], F32, tag="c1b_sb", name="c1b_sb")
        nc.vector.tensor_scalar_mul(c1b_sb, c1_sb, 0.55)

        # free-dim (row) constants broadcast across all partitions
        def row_const(pool, src, n, name):
            t = pool.tile([P, n], F32, tag=name, name=name)
            nc.gpsimd.dma_start(t, bass.AP(tensor=src, offset=0, ap=[[0, P], [1, n]]))
            return t

        # full [C, C] weight cached in SBUF as [P, NC_C, C] fp32r
        def cache_weight(pool, wten, name):
            wc = pool.tile([P, NC_C, C], F32R, tag=name, name=name)
            for ct in range(NC_C):
                raw = ld.tile([P, C], F32, tag="wc_raw", name="wc_raw", bufs=1)
                nc.sync.dma_start(raw, wten.ap()[ct * P:(ct + 1) * P, :])
                nc.gpsimd.tensor_copy(wc[:, ct, :], raw)
            return wc

        # normalize + transpose token tiles into xT[:, ct, t]
        def norm_transpose(trp, psum_tr, x_tile, xT, tcol, xn_bufs=2):
            r, negmr = _ln_stats(nc, stat, x_tile, eps_tile)
            xn = trp.tile([P, C], F32R, tag="xn", name="xn", bufs=xn_bufs)
            nc.scalar.activation(xn, x_tile, AF.Identity, bias=negmr, scale=r)
            for ct in range(NC_C):
                ps = psum_tr.tile([P, P], F32R, name="ps_tr")
                nc.tensor.transpose(ps, xn[:, ct * P:(ct + 1) * P], ident)
                nc.scalar.activation(xT[:, ct, tcol:tcol + P], ps, AF.Copy)

        # ===== Phases KV + Q (shared front-end pools) =====
        with ExitStack() as front:
            xt_res = front.enter_context(tc.tile_pool(name="xt_res", bufs=1))
            tr_in = front.enter_context(tc.tile_pool(name="tr_in", bufs=2))
            rc_kv = front.enter_context(tc.tile_pool(name="rc_kv", bufs=1))
            psum_mm = front.enter_context(
                tc.tile_pool(name="psum_kv", bufs=3, space="PSUM"))
            psum_tr = front.enter_context(
                tc.tile_pool(name="psum_kvtr", bufs=3, space="PSUM"))

            with ExitStack() as ph:
                wcache = ph.enter_context(
                    tc.tile_pool(name="wcache_kv", bufs=1))
                cv_sb = row_const(rc_kv, cv, C, "cv_sb")
                wk_c = cache_weight(wcache, wk, "wk_c")
                wv_c = cache_weight(wcache, wv, "wv_c")

                for blk in range(N // 512):
                    xpnT = xt_res.tile([P, NC_C, 512], F32R, tag="xT",
                                       name="xpnT", bufs=2)
                    for tt in range(4):
                        t = blk * 4 + tt
                        x_t = ld.tile([P, C], F32, tag="x_in", name="x_in")
                        nc.sync.dma_start(x_t, xb_t[t])
                        p_t = ld.tile([P, C], F32, tag="p_in", name="p_in")
                        nc.sync.dma_start(p_t, pos_t[t])
                        xp = tr_in.tile([P, C], F32, tag="xp", name="xp",
                                        bufs=3)
                        nc.vector.tensor_add(xp, x_t, p_t)
                        norm_transpose(tr_in, psum_tr, xp, xpnT, tt * P,
                                       xn_bufs=3)

                    # K^T[:, this block]
                    for ot in range(NC_C):
                        ps = psum_mm.tile([P, 512], F32, name="ps_mm")
                        for ct in range(NC_C):
                            nc.tensor.matmul(
                                ps, wk_c[:, ct, ot * P:(ot + 1) * P],
                                xpnT[:, ct, :],
                                start=(ct == 0), stop=(ct == NC_C - 1))
                        kev = rc_kv.tile([P, 512], F32R, tag="kev",
                                         name="kev", bufs=3)
                        nc.scalar.activation(kev, ps, AF.Identity,
                                             bias=ck_sb[:, ot:ot + 1])
                        nc.sync.dma_start(
                            ksp_t[ot][:, blk * 512:(blk + 1) * 512], kev)

                    # Vtilde rows of this block
                    for tt in range(4):
                        vrow = rc_kv.tile([P, HEADS * 65], F32R, tag="vrow",
                                          name="vrow", bufs=2)
                        vrow_r = vrow.rearrange("p (h d) -> p h d", d=65)
                        nc.vector.tensor_copy(
                            vrow_r[:, :, 64:65].rearrange("p h d -> p (h d)"),
                            ones16)
                        for ov in range(NO_C):
                            ps = psum_mm.tile([P, 512], F32, name="ps_mm")
                            for ct in range(NC_C):
                                nc.tensor.matmul(
                                    ps, xpnT[:, ct, tt * P:(tt + 1) * P],
                                    wv_c[:, ct, ov * 512:(ov + 1) * 512],
                                    start=(ct == 0), stop=(ct == NC_C - 1))
                            vsum = evict.tile([P, 512], F32, tag="ev512",
                                              name="vsum")
                            nc.vector.tensor_add(
                                vsum, ps, cv_sb[:, ov * 512:(ov + 1) * 512])
                            nc.vector.tensor_copy(
                                vrow_r[:, ov * 8:(ov + 1) * 8, 0:64],
                                vsum.rearrange("p (h d) -> p h d", d=64))
                        nc.sync.dma_start(vsp_t[blk * 4 + tt], vrow)

            # ----- Q: transpose both blocks, then ot-outer matmuls -----
            with ExitStack() as ph:
                wcache = ph.enter_context(tc.tile_pool(name="wcache_q",
                                                       bufs=1))
                wq_c = cache_weight(wcache, wq, "wq_c")
                xnTs = []
                for blk in range(TQ // 512):
                    xnT = xt_res.tile([P, NC_C, 512], F32R, tag="xT",
                                      name="xnT", bufs=2)
                    for tt in range(4):
                        t = blk * 4 + tt
                        x_t = ld.tile([P, C], F32, tag="x_in", name="x_in")
                        nc.sync.dma_start(x_t, xq_t[t])
                        norm_transpose(tr_in, psum_tr, x_t, xnT, tt * P,
                                       xn_bufs=3)
                    xnTs.append(xnT)
                for ot in range(NC_C):
                    for blk in range(TQ // 512):
                        ps = psum_mm.tile([P, 512], F32, name="ps_mm")
                        for ct in range(NC_C):
                            nc.tensor.matmul(
                                ps, wq_c[:, ct, ot * P:(ot + 1) * P],
                                xnTs[blk][:, ct, :],
                                start=(ct == 0), stop=(ct == NC_C - 1))
                        qev = rc_kv.tile([P, 512], F32R, tag="kev",
                                         name="qev", bufs=3)
                        nc.scalar.activation(qev, ps, AF.Identity,
                                             bias=cq_sb[:, ot:ot + 1])
                        nc.sync.dma_start(
                            qsp_t[ot][:, blk * 512:(blk + 1) * 512], qev)

        # h tiles live from proj to the end
        h_res = ctx.enter_context(tc.tile_pool(name="h_res", bufs=1))
        h_tiles = [h_res.tile([P, C], F32, tag=f"h{i}", name=f"h{i}")
                   for i in range(NT_Q)]

        with ExitStack() as mid:
            # attn^T stays resident through proj; wp cache preloads early
            at_res = mid.enter_context(tc.tile_pool(name="at_res", bufs=1))
            at_tiles = [at_res.tile([P, TQ], F32R, tag=f"at{i}", name=f"at{i}")
                        for i in range(NC_C)]
            wcache_p = mid.enter_context(tc.tile_pool(name="wcache_p", bufs=1))
            rc_p = mid.enter_context(tc.tile_pool(name="rc_p", bufs=1))
            cp_sb = row_const(rc_p, cp, C, "cp_sb")
            wp_c = cache_weight(wcache_p, wp, "wp_c")

            # ===== Attention =====
            with ExitStack() as pha:
                vload = pha.enter_context(tc.tile_pool(name="vload", bufs=2))
                kload = pha.enter_context(tc.tile_pool(name="kload", bufs=2))
                pexp = pha.enter_context(tc.tile_pool(name="pexp", bufs=6))
                attn_sc = pha.enter_context(
                    tc.tile_pool(name="attn_sc", bufs=4))
                psum_s = pha.enter_context(
                    tc.tile_pool(name="psum_s", bufs=3, space="PSUM"))
                psum_o = pha.enter_context(
                    tc.tile_pool(name="psum_o", bufs=2, space="PSUM"))
                for hp in range(HEADS // 2):
                    cti = hp
                    qraw = kload.tile([P, TQ], F32R, tag="qraw", name="qraw",
                                      bufs=2)
                    nc.sync.dma_start(qraw, qsp_t[cti])
                    qts = kload.tile([P, TQ], F32R, tag="qts", name="qts",
                                     bufs=2)
                    nc.vector.tensor_copy(qts, qraw)
                    vts, kts = [], []
                    for kt in range(NT_KV):
                        vraw = vload.tile([P, 130], F32R, tag="vraw",
                                          name="vraw", bufs=3)
                        nc.sync.dma_start(
                            vraw, vsp_t[kt][:, hp * 130:(hp + 1) * 130])
                        vt = vload.tile([P, 130], F32R, tag="vt", name="vt",
                                        bufs=NT_KV + 2)
                        nc.vector.tensor_copy(vt, vraw)
                        vts.append(vt)
                        kraw = kload.tile([P, P], F32R, tag="kraw",
                                          name="kraw", bufs=3)
                        nc.sync.dma_start(kraw,
                                          ksp_t[cti][:, kt * P:(kt + 1) * P])
                        ktile = kload.tile([P, P], F32R, tag="ktile",
                                           name="ktile", bufs=NT_KV + 2)
                        nc.vector.tensor_copy(ktile, kraw)
                        kts.append(ktile)
                    for qb in range(TQ // 512):
                        po = [psum_o.tile([65, 512], F32, name="po")
                              for _ in range(2)]
                        for kt in range(NT_KV):
                            ps = psum_s.tile([P, 2, 512], F32, name="ps_s")
                            for hh in range(2):
                                o2 = hh * 64
                                nc.tensor.matmul(
                                    ps[:, hh, :], kts[kt][o2:o2 + 64, :],
                                    qts[o2:o2 + 64,
                                        qb * 512:(qb + 1) * 512],
                                    start=True, stop=True)
                            pt = pexp.tile([P, 2, 512], F32R, tag="pt",
                                           name="pt")
                            nc.scalar.activation(pt, ps, AF.Exp, scale=SCALE)
                            for hh in range(2):
                                nc.tensor.matmul(
                                    po[hh],
                                    vts[kt][:, hh * 65:(hh + 1) * 65],
                                    pt[:, hh, :],
                                    start=(kt == 0), stop=(kt == NT_KV - 1))
                        for hh in range(2):
                            o2 = hh * 64
                            recip = attn_sc.tile([1, 512], F32, tag="recip",
                                                 name="recip")
                            nc.vector.reciprocal(recip, po[hh][64:65, :])
                            rb = attn_sc.tile([64, 512], F32, tag="rb",
                                              name="rb")
                            nc.gpsimd.partition_broadcast(rb, recip)
                            nc.vector.tensor_mul(
                                at_tiles[cti][o2:o2 + 64,
                                              qb * 512:(qb + 1) * 512],
                                po[hh][0:64, :], rb)

            # ===== Proj + residual -> h =====
            with ExitStack() as php:
                psum_mm = php.enter_context(
                    tc.tile_pool(name="psum_p", bufs=3, space="PSUM"))
                for tt in range(NT_Q):
                    xq_sb = ld.tile([P, C], F32, tag="x_in", name="x_in")
                    nc.sync.dma_start(xq_sb, xq_t[tt])
                    for ov in range(NO_C):
                        ps = psum_mm.tile([P, 512], F32, name="ps_mm")
                        for ct in range(NC_C):
                            nc.tensor.matmul(
                                ps, at_tiles[ct][:, tt * P:(tt + 1) * P],
                                wp_c[:, ct, ov * 512:(ov + 1) * 512],
                                start=(ct == 0), stop=(ct == NC_C - 1))
                        tmp = evict.tile([P, 512], F32, tag="ev512",
                                         name="htmp")
                        nc.vector.tensor_add(
                            tmp, ps, xq_sb[:, ov * 512:(ov + 1) * 512])
                        nc.vector.tensor_add(
                            h_tiles[tt][:, ov * 512:(ov + 1) * 512],
                            tmp, cp_sb[:, ov * 512:(ov + 1) * 512])

        # ===== MLP: single pass, y2 accumulated into h =====
        with ExitStack() as phm:
            hnt_res = phm.enter_context(tc.tile_pool(name="hnt_res", bufs=1))
            a1_res = phm.enter_context(tc.tile_pool(name="a1_res", bufs=1))
            tr_in = phm.enter_context(tc.tile_pool(name="tr_in3", bufs=2))
            w1ld = phm.enter_context(tc.tile_pool(name="w1ld", bufs=2))
            rc_m = phm.enter_context(tc.tile_pool(name="rc_m", bufs=1))
            c2_sb = row_const(rc_m, c2, C, "c2_sb")
            psum_mm = phm.enter_context(
                tc.tile_pool(name="psum_m", bufs=2, space="PSUM"))
            psum_tr = phm.enter_context(
                tc.tile_pool(name="psum_mtr", bufs=2, space="PSUM"))
            psum_y2 = phm.enter_context(
                tc.tile_pool(name="psum_y2", bufs=4, space="PSUM"))

            hnT = hnt_res.tile([P, NC_C, TQ], F32R, name="hnT")
            for tt in range(NT_Q):
                norm_transpose(tr_in, psum_tr, h_tiles[tt], hnT, tt * P,
                               xn_bufs=2)
            # h becomes the output accumulator: h += c2
            for tt in range(NT_Q):
                nc.vector.tensor_add(h_tiles[tt], h_tiles[tt], c2_sb)

            for jg in range(NJ_H // 4):
                w1c = []
                for ct in range(NC_C):
                    raw = ld.tile([P, 512], F32, tag="x_in", name="w1_raw")
                    nc.sync.dma_start(
                        raw, w1.ap()[ct * P:(ct + 1) * P,
                                     jg * 512:(jg + 1) * 512])
                    w1r = w1ld.tile([P, 512], F32R, tag="w1_r", name="w1_r",
                                    bufs=NC_C + 1)
                    nc.gpsimd.tensor_copy(w1r, raw)
                    w1c.append(w1r)
                a1g, w2g = [], []
                for j4 in range(4):
                    jt = jg * 4 + j4
                    a1t = a1_res.tile([P, TQ], F32R, tag="a1r", name="a1r",
                                      bufs=6)
                    for th in range(2):
                        ps = psum_mm.tile([P, 512], F32, name="ps_m")
                        for ct in range(NC_C):
                            nc.tensor.matmul(
                                ps, w1c[ct][:, j4 * P:(j4 + 1) * P],
                                hnT[:, ct, th * 512:(th + 1) * 512],
                                start=(ct == 0), stop=(ct == NC_C - 1))
                        # LeakyReLU(y,0.1) = 0.55*y + 0.45*|y| (exact; the
                        # HW Lrelu LUT ignores alpha and uses 0.01)
                        tabs = w1ld.tile([P, 512], F32, tag="tabs",
                                         name="tabs", bufs=2)
                        nc.scalar.activation(tabs, ps, AF.Abs, scale=0.45,
                                             bias=c1a_sb[:, jt:jt + 1])
                        t55 = w1ld.tile([P, 512], F32, tag="t55",
                                        name="t55", bufs=2)
                        nc.vector.tensor_scalar(
                            t55, ps, 0.55, c1b_sb[:, jt:jt + 1],
                            mybir.AluOpType.mult, mybir.AluOpType.add)
                        nc.vector.tensor_add(
                            a1t[:, th * 512:(th + 1) * 512], tabs, t55)
                    a1g.append(a1t)
                    raw2 = ld.tile([P, C], F32, tag="p_in", name="w2_raw")
                    nc.sync.dma_start(raw2, w2.ap()[jt * P:(jt + 1) * P, :])
                    w2r = w1ld.tile([P, C], F32R, tag="w2_r", name="w2_r",
                                    bufs=5)
                    nc.gpsimd.tensor_copy(w2r, raw2)
                    w2g.append(w2r)
                for tt in range(NT_Q):
                    for ov in range(NO_C):
                        ps2 = psum_y2.tile([P, 512], F32, name="py2")
                        for j4 in range(4):
                            nc.tensor.matmul(
                                ps2, a1g[j4][:, tt * P:(tt + 1) * P],
                                w2g[j4][:, ov * 512:(ov + 1) * 512],
                                start=(j4 == 0), stop=(j4 == 3))
                        nc.vector.tensor_add(
                            h_tiles[tt][:, ov * 512:(ov + 1) * 512], ps2,
                            h_tiles[tt][:, ov * 512:(ov + 1) * 512])
            for tt in range(NT_Q):
                nc.sync.dma_start(out.ap()[tt * P:(tt + 1) * P, :],
                                  h_tiles[tt])

    nc.compile()
    return nc


_CACHE = {}


def _get_program():
    if "nc" not in _CACHE:
        _CACHE["nc"] = build_program()
    return _CACHE["nc"]


def _get_exec():
    """Compile once; return (jitted sharded fn, metadata). Mirrors
    bass2jax.run_bass_via_pjrt but caches the executable and skips
    donation so it can be re-invoked for timing."""
    if "exec" in _CACHE:
        return _CACHE["exec"]
    import jax
    from jax.experimental.shard_map import shard_map
    from jax.sharding import Mesh, PartitionSpec
    from concourse import bass2jax, mybir as mb

    nc = _get_program()
    bass2jax.install_neuronx_cc_hook()
    partition_name = (nc.partition_id_tensor.name
                      if nc.partition_id_tensor else None)
    in_names, out_names, out_avals, zero_outs = [], [], [], []
    for alloc in nc.m.functions[0].allocations:
        if not isinstance(alloc, mb.MemoryLocationSet):
            continue
        name = alloc.memorylocations[0].name
        if alloc.kind == "ExternalInput":
            if name != partition_name:
                in_names.append(name)
        elif alloc.kind == "ExternalOutput":
            shape = tuple(alloc.tensor_shape)
            dtype = mb.dt.np(alloc.dtype)
            out_names.append(name)
            out_avals.append(jax.core.ShapedArray(shape, dtype))
            zero_outs.append(np.zeros(shape, dtype))
    n_params = len(in_names)
    all_names = list(in_names) + list(out_names)
    if partition_name is not None:
        all_names.append(partition_name)

    def _body(*args):
        operands = list(args)
        if partition_name is not None:
            operands.append(bass2jax.partition_id_tensor())
        outs = bass2jax._bass_exec_p.bind(
            *operands,
            out_avals=tuple(out_avals),
            in_names=tuple(all_names),
            out_names=tuple(out_names),
            lowering_input_output_aliases=(),
            sim_require_finite=True,
            sim_require_nnan=True,
            nc=nc,
        )
        return tuple(outs)

    devices = jax.devices()[:NCORES]
    mesh = Mesh(np.asarray(devices), ("core",))
    n_all = n_params + len(out_names)
    sharded = jax.jit(
        shard_map(_body, mesh=mesh,
                  in_specs=(PartitionSpec("core"),) * n_all,
                  out_specs=(PartitionSpec("core"),) * len(out_names),
                  check_rep=False),
        keep_unused=True,
    )
    _CACHE["exec"] = (sharded, mesh, in_names, n_params, out_names,
                      out_avals, zero_outs)
    return _CACHE["exec"]


def _run(in_maps):
    import jax
    sharded, mesh, in_names, n_params, out_names, out_avals, zero_outs =         _get_exec()
    concat_in = [
        np.concatenate([np.asarray(in_maps[c][nm]) for c in range(NCORES)],
                       axis=0)
        for nm in in_names
    ]
    concat_zeros = [
        np.zeros((NCORES * z.shape[0], *z.shape[1:]), z.dtype)
        for z in zero_outs
    ]
    out_arrs = sharded(*concat_in, *concat_zeros)
    jax.block_until_ready(out_arrs)
    return [
        {nm: np.asarray(out_arrs[i]).reshape(NCORES, *out_avals[i].shape)[c]
         for i, nm in enumerate(out_names)}
        for c in range(NCORES)
    ]


def _device_args(in_maps):
    import jax
    from jax.sharding import NamedSharding, PartitionSpec
    sharded, mesh, in_names, n_params, out_names, out_avals, zero_outs =         _get_exec()
    sh = NamedSharding(mesh, PartitionSpec("core"))
    args = [
        jax.device_put(
            np.concatenate([np.asarray(in_maps[c][nm])
                            for c in range(NCORES)], axis=0), sh)
        for nm in in_names
    ] + [
        jax.device_put(np.zeros((NCORES * z.shape[0], *z.shape[1:]), z.dtype),
                       sh)
        for z in zero_outs
    ]
    return args


def time_kernel(inputs, iters=5):
    """Marginal per-execute wall time of the compiled executable using
    pipelined async launches: (t(60) - t(10)) / 50, in ns. This subtracts
    the axon dispatch round-trip (~77 ms) that dominates a single call."""
    import time as _time
    import jax
    in_maps = _make_in_maps(**inputs)
    sharded = _get_exec()[0]
    args = _device_args(in_maps)
    jax.block_until_ready(sharded(*args))  # warm

    def run_n(n):
        best = float("inf")
        for _ in range(iters):
            t0 = _time.perf_counter()
            outs = None
            for _i in range(n):
                outs = sharded(*args)
            jax.block_until_ready(outs)
            best = min(best, _time.perf_counter() - t0)
        return best

    t10, t60 = run_n(10), run_n(60)
    return (t60 - t10) / 50.0 * 1e9


def _make_in_maps(x, pos_embed, nq_g, nq_b, nk_g, nk_b, nv_g, nv_b, wq, bq,
                  wk, bk, wv, bv, wp, bp, n_g, n_b, w1, b1, w2, b2):
    x = np.asarray(x, np.float32)
    pos = np.asarray(pos_embed, np.float32).reshape(N, C)

    def fold(g, b, w, bias):
        ws = np.asarray(g, np.float32)[:, None] * np.asarray(w, np.float32)
        cst = (np.asarray(b, np.float32) @ np.asarray(w, np.float32)
               + np.asarray(bias, np.float32))
        return np.ascontiguousarray(ws), np.ascontiguousarray(cst)

    wq_s, cq_v = fold(nq_g, nq_b, wq, bq)
    wk_s, ck_v = fold(nk_g, nk_b, wk, bk)
    wv_s, cv_v = fold(nv_g, nv_b, wv, bv)
    w1_s, c1_v = fold(n_g, n_b, w1, b1)
    wp_f = np.ascontiguousarray(np.asarray(wp, np.float32))
    w2_f = np.ascontiguousarray(np.asarray(w2, np.float32))
    cp_v = np.ascontiguousarray(np.asarray(bp, np.float32))
    c2_v = np.ascontiguousarray(np.asarray(b2, np.float32))

    in_maps = []
    for c in range(NCORES):
        b, half = divmod(c, 2)
        in_maps.append({
            "xb": np.ascontiguousarray(x[b]),
            "pos": pos,
            "xq": np.ascontiguousarray(x[b, half * TQ:(half + 1) * TQ]),
            "wq": wq_s, "wk": wk_s, "wv": wv_s, "wp": wp_f,
            "w1": w1_s, "w2": w2_f,
            "cq": cq_v, "ck": ck_v, "cv": cv_v, "cp": cp_v,
            "c1": c1_v, "c2": c2_v,
        })
    return in_maps


def kernel(**inputs):
    results = _run(_make_in_maps(**inputs))
    outa = np.empty((B, N, C), np.float32)
    for c in range(NCORES):
        b, half = divmod(c, 2)
        outa[b, half * TQ:(half + 1) * TQ] = results[c]["out"]
    return outa


# revision 40
# speedup vs baseline: 1.0849x; 1.0849x over previous
"""Trainium2 Bass kernel for a dense transformer block (pre-LN attention + MLP).

Reference computation (B=4, N=2048, C=1024, H=4096, 16 heads, fp32):
    q = LN(x) @ wq + bq ; k/v = LN(x+pos) @ w{k,v} + b{k,v}
    attn = softmax(q k^T / sqrt(hd)) @ v ; h = x + attn @ wp + bp
    out = h + leaky_relu(LN(h) @ w1 + b1, 0.1) @ w2 + b2

Sharding: 8 cores; core c handles batch c//2, query-token half c%2. K/V
for the full 2048-token sequence are recomputed per core pair, so no
collectives are needed. All matmuls run in fp32r (full PE rate at free
dim 512, ~1e-4 rms error). LayerNorm runs in token-major layout
(per-partition stats on DVE/ACT), activations are PE-transposed into
C-major layout for the projections, LN gamma is folded into the weights
host-side and LN beta + biases fold into per-output constant rows.
K^T, Vtilde (V plus a ones-column that accumulates the softmax
denominator during the P@V matmul) and attn^T stage through DRAM.
"""

import numpy as np
from contextlib import ExitStack

import concourse.bass as bass
import concourse.bacc as bacc
import concourse.tile as tile
from concourse import mybir
from concourse.bass_utils import run_bass_kernel_spmd
from concourse.masks import make_identity

F32 = mybir.dt.float32
F32R = mybir.dt.float32r
AF = mybir.ActivationFunctionType

B, N, C, H, HEADS = 4, 2048, 1024, 4096, 16
HD = C // HEADS            # 64
TQ = N // 2                # query tokens per core = 1024
EPS = 1e-5
SCALE = float(HD) ** -0.5  # 1/8
P = 128
NCORES = 8

NT_KV = N // P             # 16 token tiles (kv side)
NT_Q = TQ // P             # 8 token tiles (q side)
NC_C = C // P              # 8 channel tiles
NO_C = C // 512            # 2 output col tiles of 512
NJ_H = H // P              # 32


def _ln_stats(nc, pool, x_tile, eps_tile):
    """Return (r, negmr) = (rsqrt(var+eps), -mean*r) as [P,1] fp32 tiles."""
    stats = pool.tile([P, 2, 6], F32, tag="bn_stats", name="bn_stats")
    for sg in range(2):
        nc.vector.bn_stats(stats[:, sg, :], x_tile[:, sg * 512:(sg + 1) * 512])
    mv = pool.tile([P, 2], F32, tag="bn_mv", name="bn_mv")
    nc.vector.bn_aggr(mv, stats)
    r = pool.tile([P, 1], F32, tag="ln_r", name="ln_r")
    negmr = pool.tile([P, 1], F32, tag="ln_negmr", name="ln_negmr")
    nc.scalar.activation(r, mv[:, 1:2], AF.Sqrt, bias=eps_tile)
    nc.vector.reciprocal(r, r)
    nc.vector.tensor_mul(negmr, mv[:, 0:1], r)
    nc.vector.tensor_scalar_mul(negmr, negmr, -1.0)
    return r, negmr


def build_program():
    nc = bacc.Bacc("TRN2", target_bir_lowering=False, debug=False)

    xb = nc.dram_tensor("xb", [N, C], F32, kind="ExternalInput")
    posd = nc.dram_tensor("pos", [N, C], F32, kind="ExternalInput")
    xq = nc.dram_tensor("xq", [TQ, C], F32, kind="ExternalInput")
    wq = nc.dram_tensor("wq", [C, C], F32, kind="ExternalInput")
    wk = nc.dram_tensor("wk", [C, C], F32, kind="ExternalInput")
    wv = nc.dram_tensor("wv", [C, C], F32, kind="ExternalInput")
    wp = nc.dram_tensor("wp", [C, C], F32, kind="ExternalInput")
    w1 = nc.dram_tensor("w1", [C, H], F32, kind="ExternalInput")
    w2 = nc.dram_tensor("w2", [H, C], F32, kind="ExternalInput")
    cq = nc.dram_tensor("cq", [C], F32, kind="ExternalInput")
    ck = nc.dram_tensor("ck", [C], F32, kind="ExternalInput")
    cv = nc.dram_tensor("cv", [C], F32, kind="ExternalInput")
    cp = nc.dram_tensor("cp", [C], F32, kind="ExternalInput")
    c1 = nc.dram_tensor("c1", [H], F32, kind="ExternalInput")
    c2 = nc.dram_tensor("c2", [C], F32, kind="ExternalInput")
    out = nc.dram_tensor("out", [TQ, C], F32, kind="ExternalOutput")
    # DRAM staging (fp32r bits; re-rounded by a copy after reload)
    vsp = nc.dram_tensor("vspill", [N, HEADS * 65], F32R, kind="Internal")
    ksp = nc.dram_tensor("kspill", [C, N], F32R, kind="Internal")
    qsp = nc.dram_tensor("qspill", [C, TQ], F32R, kind="Internal")

    xb_t = xb.ap().rearrange("(t p) c -> t p c", p=P)
    pos_t = posd.ap().rearrange("(t p) c -> t p c", p=P)
    xq_t = xq.ap().rearrange("(t p) c -> t p c", p=P)
    vsp_t = vsp.ap().rearrange("(t p) c -> t p c", p=P)
    ksp_t = ksp.ap().rearrange("(ct p) t -> ct p t", p=P)
    qsp_t = qsp.ap().rearrange("(ct p) t -> ct p t", p=P)

    with tile.TileContext(nc) as tc, ExitStack() as ctx:
        const = ctx.enter_context(tc.tile_pool(name="const", bufs=1))
        stat = ctx.enter_context(tc.tile_pool(name="stat", bufs=4))
        ld = ctx.enter_context(tc.tile_pool(name="ld", bufs=2))
        evict = ctx.enter_context(tc.tile_pool(name="evict", bufs=2))

        ident_f32 = const.tile([P, P], F32)
        make_identity(nc, ident_f32)
        ident = const.tile([P, P], F32R)
        nc.vector.tensor_copy(ident, ident_f32)
        eps_tile = const.tile([P, 1], F32)
        nc.vector.memset(eps_tile, EPS)
        ones16 = const.tile([P, 16], F32)
        nc.vector.memset(ones16, 1.0)

        # per-partition bias columns
        def col_const(src, n_tiles, name):
            t = const.tile([P, n_tiles], F32, tag=name, name=name)
            nc.sync.dma_start(t, bass.AP(tensor=src, offset=0, ap=[[1, P], [P, n_tiles]]))
            return t

        cq_sb = col_const(cq, NC_C, "cq_sb")
        ck_sb = col_const(ck, NC_C, "ck_sb")
        c1_sb = col_const(c1, NJ_H, "c1_sb")
        c1a_sb = const.tile([P, NJ_H], F32, tag="c1a_sb", name="c1a_sb")
        nc.vector.tensor_scalar_mul(c1a_sb, c1_sb, 0.45)
        c1b_sb = const.tile([P, NJ_H], F32, tag="c1b_sb", name="c1b_sb")
        nc.vector.tensor_scalar_mul(c1b_sb, c1_sb, 0.55)

        # free-dim (row) constants broadcast across all partitions
        def row_const(pool, src, n, name):
            t = pool.tile([P, n], F32, tag=name, name=name)
            nc.gpsimd.dma_start(t, bass.AP(tensor=src, offset=0, ap=[[0, P], [1, n]]))
            return t

        # full [C, C] weight cached in SBUF as [P, NC_C, C] fp32r
        def cache_weight(pool, wten, name):
            wc = pool.tile([P, NC_C, C], F32R, tag=name, name=name)
            for ct in range(NC_C):
                raw = ld.tile([P, C], F32, tag="wc_raw", name="wc_raw", bufs=1)
                nc.sync.dma_start(raw, wten.ap()[ct * P:(ct + 1) * P, :])
                nc.gpsimd.tensor_copy(wc[:, ct, :], raw)
            return wc

        # normalize + transpose token tiles into xT[:, ct, t]
        def norm_transpose(trp, psum_tr, x_tile, xT, tcol, xn_bufs=2):
            r, negmr = _ln_stats(nc, stat, x_tile, eps_tile)
            xn = trp.tile([P, C], F32R, tag="xn", name="xn", bufs=xn_bufs)
            nc.scalar.activation(xn, x_tile, AF.Identity, bias=negmr, scale=r)
            for ct in range(NC_C):
                ps = psum_tr.tile([P, P], F32R, name="ps_tr")
                nc.tensor.transpose(ps, xn[:, ct * P:(ct + 1) * P], ident)
                nc.scalar.activation(xT[:, ct, tcol:tcol + P], ps, AF.Copy)

        # ===== Phases KV + Q (shared front-end pools) =====
        with ExitStack() as front:
            xt_res = front.enter_context(tc.tile_pool(name="xt_res", bufs=1))
            tr_in = front.enter_context(tc.tile_pool(name="tr_in", bufs=2))
            rc_kv = front.enter_context(tc.tile_pool(name="rc_kv", bufs=1))
            psum_mm = front.enter_context(
                tc.tile_pool(name="psum_kv", bufs=3, space="PSUM"))
            psum_tr = front.enter_context(
                tc.tile_pool(name="psum_kvtr", bufs=3, space="PSUM"))

            with ExitStack() as ph:
                wcache = ph.enter_context(
                    tc.tile_pool(name="wcache_kv", bufs=1))
                cv_sb = row_const(rc_kv, cv, C, "cv_sb")
                wk_c = cache_weight(wcache, wk, "wk_c")
                wv_c = cache_weight(wcache, wv, "wv_c")

                for blk in range(N // 512):
                    xpnT = xt_res.tile([P, NC_C, 512], F32R, tag="xT",
                                       name="xpnT", bufs=2)
                    for tt in range(4):
                        t = blk * 4 + tt
                        x_t = ld.tile([P, C], F32, tag="x_in", name="x_in")
                        nc.sync.dma_start(x_t, xb_t[t])
                        p_t = ld.tile([P, C], F32, tag="p_in", name="p_in")
                        nc.sync.dma_start(p_t, pos_t[t])
                        xp = tr_in.tile([P, C], F32, tag="xp", name="xp",
                                        bufs=3)
                        nc.vector.tensor_add(xp, x_t, p_t)
                        norm_transpose(tr_in, psum_tr, xp, xpnT, tt * P,
                                       xn_bufs=3)

                    # K^T[:, this block]
                    for ot in range(NC_C):
                        ps = psum_mm.tile([P, 512], F32, name="ps_mm")
                        for ct in range(NC_C):
                            nc.tensor.matmul(
                                ps, wk_c[:, ct, ot * P:(ot + 1) * P],
                                xpnT[:, ct, :],
                                start=(ct == 0), stop=(ct == NC_C - 1))
                        kev = rc_kv.tile([P, 512], F32R, tag="kev",
                                         name="kev", bufs=3)
                        nc.scalar.activation(kev, ps, AF.Identity,
                                             bias=ck_sb[:, ot:ot + 1])
                        nc.sync.dma_start(
                            ksp_t[ot][:, blk * 512:(blk + 1) * 512], kev)

                    # Vtilde rows of this block
                    for tt in range(4):
                        vrow = rc_kv.tile([P, HEADS * 65], F32R, tag="vrow",
                                          name="vrow", bufs=2)
                        vrow_r = vrow.rearrange("p (h d) -> p h d", d=65)
                        nc.vector.tensor_copy(
                            vrow_r[:, :, 64:65].rearrange("p h d -> p (h d)"),
                            ones16)
                        for ov in range(NO_C):
                            ps = psum_mm.tile([P, 512], F32, name="ps_mm")
                            for ct in range(NC_C):
                                nc.tensor.matmul(
                                    ps, xpnT[:, ct, tt * P:(tt + 1) * P],
                                    wv_c[:, ct, ov * 512:(ov + 1) * 512],
                                    start=(ct == 0), stop=(ct == NC_C - 1))
                            vsum = evict.tile([P, 512], F32, tag="ev512",
                                              name="vsum")
                            nc.vector.tensor_add(
                                vsum, ps, cv_sb[:, ov * 512:(ov + 1) * 512])
                            nc.vector.tensor_copy(
                                vrow_r[:, ov * 8:(ov + 1) * 8, 0:64],
                                vsum.rearrange("p (h d) -> p h d", d=64))
                        nc.sync.dma_start(vsp_t[blk * 4 + tt], vrow)

            # ----- Q: transpose both blocks, then ot-outer matmuls -----
            with ExitStack() as ph:
                wcache = ph.enter_context(tc.tile_pool(name="wcache_q",
                                                       bufs=1))
                wq_c = cache_weight(wcache, wq, "wq_c")
                xnTs = []
                for blk in range(TQ // 512):
                    xnT = xt_res.tile([P, NC_C, 512], F32R, tag="xT",
                                      name="xnT", bufs=2)
                    for tt in range(4):
                        t = blk * 4 + tt
                        x_t = ld.tile([P, C], F32, tag="x_in", name="x_in")
                        nc.sync.dma_start(x_t, xq_t[t])
                        norm_transpose(tr_in, psum_tr, x_t, xnT, tt * P,
                                       xn_bufs=3)
                    xnTs.append(xnT)
                for ot in range(NC_C):
                    for blk in range(TQ // 512):
                        ps = psum_mm.tile([P, 512], F32, name="ps_mm")
                        for ct in range(NC_C):
                            nc.tensor.matmul(
                                ps, wq_c[:, ct, ot * P:(ot + 1) * P],
                                xnTs[blk][:, ct, :],
                                start=(ct == 0), stop=(ct == NC_C - 1))
                        qev = rc_kv.tile([P, 512], F32R, tag="kev",
                                         name="qev", bufs=3)
                        nc.scalar.activation(qev, ps, AF.Identity,
                                             bias=cq_sb[:, ot:ot + 1])
                        nc.sync.dma_start(
                            qsp_t[ot][:, blk * 512:(blk + 1) * 512], qev)

        # h tiles live from proj to the end
        h_res = ctx.enter_context(tc.tile_pool(name="h_res", bufs=1))
        h_tiles = [h_res.tile([P, C], F32, tag=f"h{i}", name=f"h{i}")
                   for i in range(NT_Q)]

        with ExitStack() as mid:
            # attn^T stays resident through proj; wp cache preloads early
            at_res = mid.enter_context(tc.tile_pool(name="at_res", bufs=1))
            at_tiles = [at_res.tile([P, TQ], F32R, tag=f"at{i}", name=f"at{i}")
                        for i in range(NC_C)]
            wcache_p = mid.enter_context(tc.tile_pool(name="wcache_p", bufs=1))
            rc_p = mid.enter_context(tc.tile_pool(name="rc_p", bufs=1))
            cp_sb = row_const(rc_p, cp, C, "cp_sb")
            wp_c = cache_weight(wcache_p, wp, "wp_c")

            # ===== Attention =====
            with ExitStack() as pha:
                vload = pha.enter_context(tc.tile_pool(name="vload", bufs=2))
                kload = pha.enter_context(tc.tile_pool(name="kload", bufs=2))
                pexp = pha.enter_context(tc.tile_pool(name="pexp", bufs=6))
                attn_sc = pha.enter_context(
                    tc.tile_pool(name="attn_sc", bufs=4))
                psum_s = pha.enter_context(
                    tc.tile_pool(name="psum_s", bufs=2, space="PSUM"))
                psum_o = pha.enter_context(
                    tc.tile_pool(name="psum_o", bufs=4, space="PSUM"))
                for hp in range(HEADS // 2):
                    cti = hp
                    qraw = kload.tile([P, TQ], F32R, tag="qraw", name="qraw",
                                      bufs=2)
                    nc.sync.dma_start(qraw, qsp_t[cti])
                    qts = kload.tile([P, TQ], F32R, tag="qts", name="qts",
                                     bufs=2)
                    nc.vector.tensor_copy(qts, qraw)
                    vts, kts = [], []
                    for kt in range(NT_KV):
                        vraw = vload.tile([P, 130], F32R, tag="vraw",
                                          name="vraw", bufs=3)
                        nc.sync.dma_start(
                            vraw, vsp_t[kt][:, hp * 130:(hp + 1) * 130])
                        vt = vload.tile([P, 130], F32R, tag="vt", name="vt",
                                        bufs=NT_KV + 2)
                        nc.vector.tensor_copy(vt, vraw)
                        vts.append(vt)
                        kraw = kload.tile([P, P], F32R, tag="kraw",
                                          name="kraw", bufs=3)
                        nc.sync.dma_start(kraw,
                                          ksp_t[cti][:, kt * P:(kt + 1) * P])
                        ktile = kload.tile([P, P], F32R, tag="ktile",
                                           name="ktile", bufs=NT_KV + 2)
                        nc.vector.tensor_copy(ktile, kraw)
                        kts.append(ktile)
                    for qb in range(TQ // 512):
                        po = [psum_o.tile([65, 512], F32, name="po")
                              for _ in range(2)]
                        for kt in range(NT_KV):
                            ps = psum_s.tile([P, 2, 512], F32, name="ps_s")
                            for hh in range(2):
                                o2 = hh * 64
                                nc.tensor.matmul(
                                    ps[:, hh, :], kts[kt][o2:o2 + 64, :],
                                    qts[o2:o2 + 64,
                                        qb * 512:(qb + 1) * 512],
                                    start=True, stop=True)
                            pt = pexp.tile([P, 2, 512], F32R, tag="pt",
                                           name="pt")
                            nc.scalar.activation(pt, ps, AF.Exp, scale=SCALE)
                            for hh in range(2):
                                nc.tensor.matmul(
                                    po[hh],
                                    vts[kt][:, hh * 65:(hh + 1) * 65],
                                    pt[:, hh, :],
                                    start=(kt == 0), stop=(kt == NT_KV - 1))
                        for hh in range(2):
                            o2 = hh * 64
                            recip = attn_sc.tile([1, 512], F32, tag="recip",
                                                 name="recip")
                            nc.vector.reciprocal(recip, po[hh][64:65, :])
                            rb = attn_sc.tile([64, 512], F32, tag="rb",
                                              name="rb")
                            nc.gpsimd.partition_broadcast(rb, recip)
                            nc.vector.tensor_mul(
                                at_tiles[cti][o2:o2 + 64,
                                              qb * 512:(qb + 1) * 512],
                                po[hh][0:64, :], rb)

            # ===== Proj + residual -> h =====
            with ExitStack() as php:
                psum_mm = php.enter_context(
                    tc.tile_pool(name="psum_p", bufs=3, space="PSUM"))
                for tt in range(NT_Q):
                    xq_sb = ld.tile([P, C], F32, tag="x_in", name="x_in")
                    nc.sync.dma_start(xq_sb, xq_t[tt])
                    for ov in range(NO_C):
                        ps = psum_mm.tile([P, 512], F32, name="ps_mm")
                        for ct in range(NC_C):
                            nc.tensor.matmul(
                                ps, at_tiles[ct][:, tt * P:(tt + 1) * P],
                                wp_c[:, ct, ov * 512:(ov + 1) * 512],
                                start=(ct == 0), stop=(ct == NC_C - 1))
                        tmp = evict.tile([P, 512], F32, tag="ev512",
                                         name="htmp")
                        nc.vector.tensor_add(
                            tmp, ps, xq_sb[:, ov * 512:(ov + 1) * 512])
                        nc.vector.tensor_add(
                            h_tiles[tt][:, ov * 512:(ov + 1) * 512],
                            tmp, cp_sb[:, ov * 512:(ov + 1) * 512])

        # ===== MLP: single pass, y2 accumulated into h =====
        with ExitStack() as phm:
            hnt_res = phm.enter_context(tc.tile_pool(name="hnt_res", bufs=1))
            a1_res = phm.enter_context(tc.tile_pool(name="a1_res", bufs=1))
            tr_in = phm.enter_context(tc.tile_pool(name="tr_in3", bufs=2))
            w1ld = phm.enter_context(tc.tile_pool(name="w1ld", bufs=2))
            rc_m = phm.enter_context(tc.tile_pool(name="rc_m", bufs=1))
            c2_sb = row_const(rc_m, c2, C, "c2_sb")
            psum_mm = phm.enter_context(
                tc.tile_pool(name="psum_m", bufs=2, space="PSUM"))
            psum_tr = phm.enter_context(
                tc.tile_pool(name="psum_mtr", bufs=2, space="PSUM"))
            psum_y2 = phm.enter_context(
                tc.tile_pool(name="psum_y2", bufs=4, space="PSUM"))

            hnT = hnt_res.tile([P, NC_C, TQ], F32R, name="hnT")
            for tt in range(NT_Q):
                norm_transpose(tr_in, psum_tr, h_tiles[tt], hnT, tt * P,
                               xn_bufs=2)
            # h becomes the output accumulator: h += c2
            for tt in range(NT_Q):
                nc.vector.tensor_add(h_tiles[tt], h_tiles[tt], c2_sb)

            for jg in range(NJ_H // 4):
                w1c = []
                for ct in range(NC_C):
                    raw = ld.tile([P, 512], F32, tag="x_in", name="w1_raw")
                    nc.sync.dma_start(
                        raw, w1.ap()[ct * P:(ct + 1) * P,
                                     jg * 512:(jg + 1) * 512])
                    w1r = w1ld.tile([P, 512], F32R, tag="w1_r", name="w1_r",
                                    bufs=NC_C + 1)
                    nc.gpsimd.tensor_copy(w1r, raw)
                    w1c.append(w1r)
                a1g, w2g = [], []
                for j4 in range(4):
                    jt = jg * 4 + j4
                    a1t = a1_res.tile([P, TQ], F32R, tag="a1r", name="a1r",
                                      bufs=6)
                    for th in range(2):
                        ps = psum_mm.tile([P, 512], F32, name="ps_m")
                        for ct in range(NC_C):
                            nc.tensor.matmul(
                                ps, w1c[ct][:, j4 * P:(j4 + 1) * P],
                                hnT[:, ct, th * 512:(th + 1) * 512],
                                start=(ct == 0), stop=(ct == NC_C - 1))
                        # LeakyReLU(y,0.1) = 0.55*y + 0.45*|y| (exact; the
                        # HW Lrelu LUT ignores alpha and uses 0.01)
                        tabs = w1ld.tile([P, 512], F32, tag="tabs",
                                         name="tabs", bufs=2)
                        nc.scalar.activation(tabs, ps, AF.Abs, scale=0.45,
                                             bias=c1a_sb[:, jt:jt + 1])
                        t55 = w1ld.tile([P, 512], F32, tag="t55",
                                        name="t55", bufs=2)
                        nc.vector.tensor_scalar(
                            t55, ps, 0.55, c1b_sb[:, jt:jt + 1],
                            mybir.AluOpType.mult, mybir.AluOpType.add)
                        nc.vector.tensor_add(
                            a1t[:, th * 512:(th + 1) * 512], tabs, t55)
                    a1g.append(a1t)
                    raw2 = ld.tile([P, C], F32, tag="p_in", name="w2_raw")
                    nc.sync.dma_start(raw2, w2.ap()[jt * P:(jt + 1) * P, :])
                    w2r = w1ld.tile([P, C], F32R, tag="w2_r", name="w2_r",
                                    bufs=5)
                    nc.gpsimd.tensor_copy(w2r, raw2)
                    w2g.append(w2r)
                for tt in range(NT_Q):
                    for ov in range(NO_C):
                        ps2 = psum_y2.tile([P, 512], F32, name="py2")
                        for j4 in range(4):
                            nc.tensor.matmul(
                                ps2, a1g[j4][:, tt * P:(tt + 1) * P],
                                w2g[j4][:, ov * 512:(ov + 1) * 512],
                                start=(j4 == 0), stop=(j4 == 3))
                        nc.vector.tensor_add(
                            h_tiles[tt][:, ov * 512:(ov + 1) * 512], ps2,
                            h_tiles[tt][:, ov * 512:(ov + 1) * 512])
            for tt in range(NT_Q):
                nc.sync.dma_start(out.ap()[tt * P:(tt + 1) * P, :],
                                  h_tiles[tt])

    nc.compile()
    return nc


_CACHE = {}


def _get_program():
    if "nc" not in _CACHE:
        _CACHE["nc"] = build_program()
    return _CACHE["nc"]


def _get_exec():
    """Compile once; return (jitted sharded fn, metadata). Mirrors
    bass2jax.run_bass_via_pjrt but caches the executable and skips
    donation so it can be re-invoked for timing."""
    if "exec" in _CACHE:
        return _CACHE["exec"]
    import jax
    from jax.experimental.shard_map import shard_map
    from jax.sharding import Mesh, PartitionSpec
    from concourse import bass2jax, mybir as mb

    nc = _get_program()
    bass2jax.install_neuronx_cc_hook()
    partition_name = (nc.partition_id_tensor.name
                      if nc.partition_id_tensor else None)
    in_names, out_names, out_avals, zero_outs = [], [], [], []
    for alloc in nc.m.functions[0].allocations:
        if not isinstance(alloc, mb.MemoryLocationSet):
            continue
        name = alloc.memorylocations[0].name
        if alloc.kind == "ExternalInput":
            if name != partition_name:
                in_names.append(name)
        elif alloc.kind == "ExternalOutput":
            shape = tuple(alloc.tensor_shape)
            dtype = mb.dt.np(alloc.dtype)
            out_names.append(name)
            out_avals.append(jax.core.ShapedArray(shape, dtype))
            zero_outs.append(np.zeros(shape, dtype))
    n_params = len(in_names)
    all_names = list(in_names) + list(out_names)
    if partition_name is not None:
        all_names.append(partition_name)

    def _body(*args):
        operands = list(args)
        if partition_name is not None:
            operands.append(bass2jax.partition_id_tensor())
        outs = bass2jax._bass_exec_p.bind(
            *operands,
            out_avals=tuple(out_avals),
            in_names=tuple(all_names),
            out_names=tuple(out_names),
            lowering_input_output_aliases=(),
            sim_require_finite=True,
            sim_require_nnan=True,
            nc=nc,
        )
        return tuple(outs)

    devices = jax.devices()[:NCORES]
    mesh = Mesh(np.asarray(devices), ("core",))
    n_all = n_params + len(out_names)
    sharded = jax.jit(
        shard_map(_body, mesh=mesh,
                  in_specs=(PartitionSpec("core"),) * n_all,
                  out_specs=(PartitionSpec("core"),) * len(out_names),
                  check_rep=False),
        keep_unused=True,
    )
    _CACHE["exec"] = (sharded, mesh, in_names, n_params, out_names,
                      out_avals, zero_outs)
    return _CACHE["exec"]


def _run(in_maps):
    import jax
    sharded, mesh, in_names, n_params, out_names, out_avals, zero_outs =         _get_exec()
    concat_in = [
        np.concatenate([np.asarray(in_maps[c][nm]) for c in range(NCORES)],
                       axis=0)
        for nm in in_names
    ]
    concat_zeros = [
        np.zeros((NCORES * z.shape[0], *z.shape[1:]), z.dtype)
        for z in zero_outs
    ]
    out_arrs = sharded(*concat_in, *concat_zeros)
    jax.block_until_ready(out_arrs)
    return [
        {nm: np.asarray(out_arrs[i]).reshape(NCORES, *out_avals[i].shape)[c]
         for i, nm in enumerate(out_names)}
        for c in range(NCORES)
    ]


def _device_args(in_maps):
    import jax
    from jax.sharding import NamedSharding, PartitionSpec
    sharded, mesh, in_names, n_params, out_names, out_avals, zero_outs =         _get_exec()
    sh = NamedSharding(mesh, PartitionSpec("core"))
    args = [
        jax.device_put(
            np.concatenate([np.asarray(in_maps[c][nm])
                            for c in range(NCORES)], axis=0), sh)
        for nm in in_names
    ] + [
        jax.device_put(np.zeros((NCORES * z.shape[0], *z.shape[1:]), z.dtype),
                       sh)
        for z in zero_outs
    ]
    return args


def time_kernel(inputs, iters=5):
    """Marginal per-execute wall time of the compiled executable using
    pipelined async launches: (t(60) - t(10)) / 50, in ns. This subtracts
    the axon dispatch round-trip (~77 ms) that dominates a single call."""
    import time as _time
    import jax
    in_maps = _make_in_maps(**inputs)
    sharded = _get_exec()[0]
    args = _device_args(in_maps)
    jax.block_until_ready(sharded(*args))  # warm

    def run_n(n):
        best = float("inf")
        for _ in range(iters):
            t0 = _time.perf_counter()
            outs = None
            for _i in range(n):
                outs = sharded(*args)
            jax.block_until_ready(outs)
            best = min(best, _time.perf_counter() - t0)
        return best

    t10, t60 = run_n(10), run_n(60)
    return (t60 - t10) / 50.0 * 1e9


def _make_in_maps(x, pos_embed, nq_g, nq_b, nk_g, nk_b, nv_g, nv_b, wq, bq,
                  wk, bk, wv, bv, wp, bp, n_g, n_b, w1, b1, w2, b2):
    x = np.asarray(x, np.float32)
    pos = np.asarray(pos_embed, np.float32).reshape(N, C)

    def fold(g, b, w, bias):
        ws = np.asarray(g, np.float32)[:, None] * np.asarray(w, np.float32)
        cst = (np.asarray(b, np.float32) @ np.asarray(w, np.float32)
               + np.asarray(bias, np.float32))
        return np.ascontiguousarray(ws), np.ascontiguousarray(cst)

    wq_s, cq_v = fold(nq_g, nq_b, wq, bq)
    wk_s, ck_v = fold(nk_g, nk_b, wk, bk)
    wv_s, cv_v = fold(nv_g, nv_b, wv, bv)
    w1_s, c1_v = fold(n_g, n_b, w1, b1)
    wp_f = np.ascontiguousarray(np.asarray(wp, np.float32))
    w2_f = np.ascontiguousarray(np.asarray(w2, np.float32))
    cp_v = np.ascontiguousarray(np.asarray(bp, np.float32))
    c2_v = np.ascontiguousarray(np.asarray(b2, np.float32))

    in_maps = []
    for c in range(NCORES):
        b, half = divmod(c, 2)
        in_maps.append({
            "xb": np.ascontiguousarray(x[b]),
            "pos": pos,
            "xq": np.ascontiguousarray(x[b, half * TQ:(half + 1) * TQ]),
            "wq": wq_s, "wk": wk_s, "wv": wv_s, "wp": wp_f,
            "w1": w1_s, "w2": w2_f,
            "cq": cq_v, "ck": ck_v, "cv": cv_v, "cp": cp_v,
            "c1": c1_v, "c2": c2_v,
        })
    return in_maps


def kernel(**inputs):
    results = _run(_make_in_maps(**inputs))
    outa = np.empty((B, N, C), np.float32)
    for c in range(NCORES):
        b, half = divmod(c, 2)
        outa[b, half * TQ:(half + 1) * TQ] = results[c]["out"]
    return outa
